# revision 5
# baseline (speedup 1.0000x reference)
"""Additive (Bahdanau) attention kernel for 8 Trainium2 NeuronCores.

Problem shapes (hardcoded): B=64, T=2048, Q_DIM=K_DIM=H_DIM=1024.
  q_proj = query @ Wq + bq                      (B, H)
  k_proj = keys @ Wk + bk                       (B, T, H)
  energy = tanh(q_proj[:, None, :] + k_proj)    (B, T, H)
  scores = energy @ Wv + bv                     (B, T)
  attn   = softmax(scores, axis=1)              (B, T)
  context= attn @ values                        (B, V)
returns (context, attn).

Strategy: data-parallel over batch, 8 batches per core.
 - Host: fold q_proj+bq+bk into a per-(batch,h) bias (fp32, exact); bv
   drops out of the softmax entirely. Cast keys/values/Wk/Wv to bf16 and
   pre-transpose keys to [K, T] per batch so the device only does clean
   contiguous DMAs.
 - Device per batch: k_proj via PE matmuls with Wk stationary
   ([h partitions, t free] orientation, fp32 PSUM accum over K chunks);
   tanh+bias fused in one ScalarE activation per tile (bias is
   per-partition = per-h); scores via Wv-stationary matmul (contract h on
   partitions); softmax on the free dim; context via attn-stationary
   matmul over values in native [t, v] layout.
"""

import os
import sys
from contextlib import ExitStack

for _p in ("/opt/trn_rl_repo", "/root/.axon_site/_ro/trn_rl_repo"):
    if os.path.isdir(_p) and _p not in sys.path:
        sys.path.append(_p)

import numpy as np
import ml_dtypes

BF16 = ml_dtypes.bfloat16

N_CORES = 8
B, T, H, KD = 64, 2048, 1024, 1024
NB = B // N_CORES          # batches per core
KO = KD // 128             # contraction chunks (k on partitions)
HT = H // 128              # h tiles (output partitions)
NT = 512                   # matmul moving free dim
TT = T // NT               # t tiles per batch
TC = T // 128              # t chunks (contraction for context)

_CACHE = {}


def _build():
    """Build + bacc-compile the SPMD Bass program (once per process)."""
    import concourse.bacc as bacc
    import concourse.tile as tile
    from concourse import mybir

    f32 = mybir.dt.float32
    bf16 = mybir.dt.bfloat16
    Tanh = mybir.ActivationFunctionType.Tanh
    Exp = mybir.ActivationFunctionType.Exp

    nc = bacc.Bacc("TRN2", target_bir_lowering=False, debug=False,
                   num_devices=N_CORES)

    kT_d = nc.dram_tensor("kT", [NB, KD, T], bf16, kind="ExternalInput").ap()
    val_d = nc.dram_tensor("vals", [NB, T, H], bf16, kind="ExternalInput").ap()
    wk_d = nc.dram_tensor("wk", [KD, H], bf16, kind="ExternalInput").ap()
    qbT_d = nc.dram_tensor("qbT", [128, NB * HT], f32, kind="ExternalInput").ap()
    wvT_d = nc.dram_tensor("wvT", [128, HT], bf16, kind="ExternalInput").ap()
    ctx_d = nc.dram_tensor("ctx", [NB, H], f32, kind="ExternalOutput").ap()
    attn_d = nc.dram_tensor("attn", [NB, T], f32, kind="ExternalOutput").ap()
    attnbf_d = nc.dram_tensor("attnbf", [NB, T], bf16).ap()  # internal scratch

    with tile.TileContext(nc) as tc, ExitStack() as ctx:
        const = ctx.enter_context(tc.tile_pool(name="const", bufs=1))
        kt_pool = ctx.enter_context(tc.tile_pool(name="kt", bufs=2))
        val_pool = ctx.enter_context(tc.tile_pool(name="val", bufs=2))
        en_pool = ctx.enter_context(tc.tile_pool(name="en", bufs=2))
        sc_pool = ctx.enter_context(tc.tile_pool(name="sc", bufs=2))
        at_pool = ctx.enter_context(tc.tile_pool(name="at", bufs=3))
        sm_pool = ctx.enter_context(tc.tile_pool(name="sm", bufs=2))
        vec_pool = ctx.enter_context(tc.tile_pool(name="vec", bufs=1))
        kp_psum = ctx.enter_context(tc.tile_pool(name="kp", bufs=4, space="PSUM"))
        vp_psum = ctx.enter_context(tc.tile_pool(name="vp", bufs=4, space="PSUM"))

        # resident constants
        wk_sb = const.tile([128, KO, H], bf16, tag="wk")
        nc.sync.dma_start(wk_sb[:], wk_d.rearrange("(ko p) h -> p ko h", p=128))
        qbT_sb = const.tile([128, NB * HT], f32, tag="qbT")
        nc.sync.dma_start(qbT_sb[:], qbT_d[:])
        wvT_sb = const.tile([128, HT], bf16, tag="wvT")
        nc.sync.dma_start(wvT_sb[:], wvT_d[:])

        saved = {}  # b -> (val_tile, attnT_tile)

        def emit_ctx(b):
            val_t, aT = saved.pop(b)
            cp0 = vp_psum.tile([1, NT], f32, tag="vp")
            cp1 = vp_psum.tile([1, NT], f32, tag="vp")
            for t in range(TC):
                nc.tensor.matmul(cp0[:], aT[:, t:t + 1], val_t[:, t, 0:NT],
                                 start=(t == 0), stop=(t == TC - 1))
                nc.tensor.matmul(cp1[:], aT[:, t:t + 1], val_t[:, t, NT:H],
                                 start=(t == 0), stop=(t == TC - 1))
            cs = vec_pool.tile([1, H], f32, tag="cs")
            nc.scalar.copy(cs[:, 0:NT], cp0[:])
            nc.scalar.copy(cs[:, NT:H], cp1[:])
            nc.sync.dma_start(ctx_d[b:b + 1, :], cs[:])

        for b in range(NB):
            kt_t = kt_pool.tile([128, KO, T], bf16, tag="kt")
            nc.sync.dma_start(kt_t[:], kT_d[b].rearrange("(ko p) t -> p ko t", p=128))
            val_t = val_pool.tile([128, TC, H], bf16, tag="val")
            nc.sync.dma_start(val_t[:], val_d[b].rearrange("(tc p) v -> p tc v", p=128))

            sc_sb = sc_pool.tile([1, T], f32, tag="sc")
            for tt in range(TT):
                en_t = en_pool.tile([128, HT, NT], bf16, tag="en")
                for ht in range(HT):
                    kp = kp_psum.tile([128, NT], f32, tag="kp")
                    for ko in range(KO):
                        nc.tensor.matmul(
                            kp[:],
                            wk_sb[:, ko, ht * 128:(ht + 1) * 128],
                            kt_t[:, ko, tt * NT:(tt + 1) * NT],
                            start=(ko == 0), stop=(ko == KO - 1))
                    nc.scalar.activation(
                        en_t[:, ht, :], kp[:], Tanh,
                        bias=qbT_sb[:, b * HT + ht:b * HT + ht + 1])
                sp = vp_psum.tile([1, NT], f32, tag="vp")
                for ht in range(HT):
                    nc.tensor.matmul(sp[:], wvT_sb[:, ht:ht + 1], en_t[:, ht, :],
                                     start=(ht == 0), stop=(ht == HT - 1))
                nc.scalar.copy(sc_sb[:, tt * NT:(tt + 1) * NT], sp[:])

            # context for the previous batch lands here on PE, after this
            # batch's k_proj/scores — hides the softmax/attnT latency.
            if b > 0:
                emit_ctx(b - 1)

            # softmax over t (free dim, single partition)
            mx = sm_pool.tile([1, 1], f32, tag="mx")
            nc.vector.reduce_max(mx[:], sc_sb[:], axis=mybir.AxisListType.X)
            nmx = sm_pool.tile([1, 1], f32, tag="nmx")
            nc.vector.tensor_scalar_mul(nmx[:], mx[:], -1.0)
            ssum = sm_pool.tile([1, 1], f32, tag="ssum")
            nc.scalar.activation(sc_sb[:], sc_sb[:], Exp, bias=nmx[:],
                                 accum_out=ssum[:])
            rs = sm_pool.tile([1, 1], f32, tag="rs")
            nc.vector.reciprocal(rs[:], ssum[:])
            att_f = vec_pool.tile([1, T], f32, tag="attf")
            nc.vector.tensor_scalar_mul(att_f[:], sc_sb[:], rs[:])
            nc.sync.dma_start(attn_d[b:b + 1, :], att_f[:])
            att_b = vec_pool.tile([1, T], bf16, tag="attb")
            nc.vector.tensor_scalar_mul(att_b[:], sc_sb[:], rs[:])
            nc.sync.dma_start(attnbf_d[b:b + 1, :], att_b[:])
            # transpose attn to [t partitions, chunk] via DRAM roundtrip
            aT = at_pool.tile([128, TC], bf16, tag="aT")
            with nc.allow_non_contiguous_dma(reason="tiny 4KB attn transpose"):
                nc.gpsimd.dma_start(aT[:], attnbf_d[b].rearrange("(tc p) -> p tc", p=128))
            saved[b] = (val_t, aT)

        emit_ctx(NB - 1)

    nc.compile()
    return nc


def _prep(query, keys, values, Wq, bq, Wk, bk, Wv, bv):
    qb = (query.astype(np.float32) @ Wq.astype(np.float32)
          + bq.astype(np.float32) + bk.astype(np.float32))       # [B, H]
    wk_bf = np.ascontiguousarray(Wk.astype(BF16))
    wvT = np.ascontiguousarray(Wv.reshape(HT, 128).T.astype(BF16))
    in_maps = []
    for c in range(N_CORES):
        sl = slice(c * NB, (c + 1) * NB)
        kT = np.ascontiguousarray(
            keys[sl].transpose(0, 2, 1).astype(BF16))            # [NB, K, T]
        vals = np.ascontiguousarray(values[sl].astype(BF16))     # [NB, T, H]
        qbT = np.ascontiguousarray(
            qb[sl].reshape(NB, HT, 128).transpose(2, 0, 1).reshape(128, NB * HT))
        in_maps.append({"kT": kT, "vals": vals, "wk": wk_bf,
                        "qbT": qbT, "wvT": wvT})
    return in_maps


def kernel(query, keys, values, Wq, bq, Wk, bk, Wv, bv):
    from concourse.bass_utils import run_bass_kernel_spmd

    if "nc" not in _CACHE:
        _CACHE["nc"] = _build()
    nc = _CACHE["nc"]

    in_maps = _prep(query, keys, values, Wq, bq, Wk, bk, Wv, bv)
    trace = bool(int(os.environ.get("KERNEL_TRACE", "0")))
    res = run_bass_kernel_spmd(nc, in_maps, core_ids=list(range(N_CORES)),
                               trace=trace)
    _CACHE["last_exec_ns"] = res.exec_time_ns
    _CACHE["last_results"] = res

    context = np.concatenate([res.results[c]["ctx"] for c in range(N_CORES)], axis=0)
    attn = np.concatenate([res.results[c]["attn"] for c in range(N_CORES)], axis=0)
    return context.astype(np.float32), attn.astype(np.float32)
